# revision 31
# baseline (speedup 1.0000x reference)
"""Trainium2 Bass kernel for 2-layer GCN forward (Reddit-like), 8-way node-sharded.

Design (v6; 3.32ms baseline -> ~1.23ms):
- Nodes partitioned contiguously across 8 cores (12500 each); edges routed to
  the core owning their destination, grouped into (4-tile dst group x source
  bank) cells and packed densely into 128-edge chunks (chunks may straddle
  tile boundaries; per-(chunk,tile) one-hot segments handle the split).
- bf16 everywhere (PSUM accumulates fp32): gather tables are [*, 128]-bf16
  rows (256B = dma_gather elem granularity); only the low 64 cols are written
  or read, the upper 128B per row is dead weight forced by the granularity.
- Per-edge gathers ride 4 SWDGE queues in parallel (queue = (bank+group)%4);
  Q7 descriptor emission parallelizes almost perfectly across queues
  (8.3 -> ~2.1 ns/idx aggregate; queues > 4 are rejected by ucode). With all
  8 cores gathering, the layer walls are bound by HBM random-256B-read drain
  (~75-90 GB/s/core), slightly above the emission floor.
- Banks are (tile-slice A/B x core-half), each < 32768 rows for the int16
  idx; each layer's table AllGathers in two slices so A-bank gathers start
  under phase-A's second half + AllGather-B; two-group-ahead A prefetch
  (gpool bufs=3) keeps queues fed across the slice boundary.
- Phase A (x @ W1) runs transposed with W1-stationary [128,64] lhsT over
  512-node rhs blocks, then PE-transposes back per tile (5x fewer PE instrs
  than the natural orientation); phase D (relu(h1) @ W2) is folded per-tile
  into the conv1 group loop so the layer-2 table slices AllGather under the
  layer-1 gather tail (AG2A issues mid-conv1).
- Factorized GCN norm: table rows pre-scaled by dinv[src]; dinv[dst] applied
  post-aggregation. Self-loops are descriptor-free identity matmuls from the
  SBUF-resident local shard (gloc, overwritten in place by folded phase D to
  become the layer-2 table).
- Slots within each (cell, tile) run are sorted by source address (neutral on
  drain in measurement, kept for DRAM-scheduler friendliness).
- single_packet=True and negative idxs crash the device; padding slots gather
  row 0 and their one-hot columns are zeroed (erel = -1), with per-call
  num_idxs trimmed to the max real cell count across cores. Collectives must
  stay on gpsimd (walrus rejects other trigger engines) and need contiguous
  outs. Dead ends measured: conv1 split into per-slice passes (pass-boundary
  overhead eats the overlap win), static queue=bank, 3-deep A prefetch,
  1-ahead B prefetch (head-of-line blocks A during the AG-B wait).
"""
import numpy as np
import ml_dtypes
from contextlib import ExitStack

BF16 = ml_dtypes.bfloat16

import concourse.bass as bass
import concourse.bacc as bacc
import concourse.tile as tile
from concourse import mybir
from concourse.bass_utils import run_bass_kernel_spmd

N = 100000
E = 1250000
F_IN = 602
F_PAD = 640          # 5 x 128
HID = 64
C = 41
N_CORES = 8
NT = N // N_CORES    # 12500
P = 128
N_TILES = (NT + P - 1) // P          # 98
NTP = N_TILES * P                    # 12544 (padded node cols)
TW = 128                             # table row width (bf16) -> 256B rows
GROUP = 4                            # dst tiles per gather group
N_BANKS = 4
TILES_A = 48                         # tiles in slice A (rows 0..6144)
SZA = TILES_A * P                    # 6144 rows per core in slice A
TILES_B = N_TILES - TILES_A          # 50 tiles (last partial)
SZB = 52 * P - SZA + SZA            # pad B to block multiple: tiles 48..99
SZB = (100 - TILES_A) * P            # 6656 rows per core in slice B (padded)
BANK_SZ = [4 * SZA, 4 * SZA, 4 * SZB, 4 * SZB]   # 24576, 24576, 26624, 26624
NBLK = 25                            # phase-A blocks of 4 tiles (tiles 0..99)

SINGLE_PACKET = False
PROFILE = False
_LAST_RESULTS = {}


def _src_bank(src):
    """bank + in-bank offset for source nodes under the A/B slice layout."""
    c = src // NT
    r = src % NT
    in_a = r < SZA
    bank = np.where(in_a, np.where(c < 4, 0, 1), np.where(c < 4, 2, 3))
    off = np.where(in_a, (c % 4) * SZA + r, (c % 4) * SZB + (r - SZA))
    return bank.astype(np.int64), off.astype(np.int64)


def _preprocess(x, src, dst, W1, b1, W2, b2):
    src = np.asarray(src).astype(np.int64).ravel()
    dst = np.asarray(dst).astype(np.int64).ravel()
    x = np.asarray(x, dtype=np.float32)
    W1 = np.asarray(W1, dtype=np.float32)
    b1 = np.asarray(b1, dtype=np.float32)
    W2 = np.asarray(W2, dtype=np.float32)
    b2 = np.asarray(b2, dtype=np.float32)

    deg = (np.bincount(dst, minlength=N) + 1.0).astype(np.float32)
    dinv = (1.0 / np.sqrt(deg)).astype(np.float32)

    core = dst // NT
    trel = (dst % NT) // P
    bank, bloc_all = _src_bank(src)
    n_groups = (N_TILES + GROUP - 1) // GROUP
    groups = [list(range(g, min(g + GROUP, N_TILES))) for g in range(0, N_TILES, GROUP)]
    gid = trel // GROUP
    # sort edges by (core, group, bank, tile, src offset): ascending source
    # addresses within each run give the SDMA engines a monotone stream
    key = ((((core * n_groups + gid) * N_BANKS + bank) * N_TILES + trel) * (1 << 15)
           + bloc_all).astype(np.int64)
    order = np.argsort(key, kind="stable")
    s_o = src[order]
    d_o = dst[order]
    core_o = core[order]
    gid_o = gid[order]
    bank_o = bank[order]
    bloc_o = bloc_all[order]
    trel_o = trel[order]

    # per (core, group, bank) cell counts
    cellkey = ((core_o * n_groups + gid_o) * N_BANKS + bank_o)
    n_cells = N_CORES * n_groups * N_BANKS
    cell_counts = np.bincount(cellkey, minlength=n_cells).reshape(
        N_CORES, n_groups, N_BANKS
    )
    maxcnt = cell_counts.max(axis=0)                     # [n_groups, N_BANKS]
    cw_cell = np.ceil(maxcnt / P).astype(np.int64)       # chunks per cell
    NCH = int(cw_cell.sum())

    # canonical chunk order: group -> bank -> chunks; per-group chunk offsets
    ch0_cell = np.zeros((n_groups, N_BANKS), dtype=np.int64)
    group_start = []
    group_cw = []
    call_plan = []   # (gidx, bank, ch0, cw_chunks, num_idxs)
    cursor = 0
    for gi in range(n_groups):
        group_start.append(cursor)
        for b in range(N_BANKS):
            ch0_cell[gi, b] = cursor
            cw = int(cw_cell[gi, b])
            if cw:
                call_plan.append((gi, b, cursor, cw, int(maxcnt[gi, b])))
            cursor += cw
        group_cw.append(cursor - group_start[-1])
    assert cursor == NCH
    CWG_MAX = max(group_cw)

    # per-core per-(cell, tile) slot ranges -> union segment list
    tilekey = ((core_o * n_groups + gid_o) * N_BANKS + bank_o) * N_TILES + trel_o
    tcnt = np.bincount(tilekey, minlength=n_cells * N_TILES).reshape(
        N_CORES, n_groups, N_BANKS, N_TILES
    )
    seg_set = {}
    for gi, g in enumerate(groups):
        for b in range(N_BANKS):
            for c in range(N_CORES):
                off = 0
                for t in g:
                    n = int(tcnt[c, gi, b, t])
                    if n:
                        for ch in range(off // P, (off + n - 1) // P + 1):
                            seg_set[(gi, b, ch, t)] = True
                    off += n
    segs = sorted(seg_set.keys())
    NSEG = len(segs)
    seg_index = {s: i for i, s in enumerate(segs)}
    seg_start_g = [0] * (n_groups + 1)
    segs_by_group = [[] for _ in range(n_groups)]
    for i, s in enumerate(segs):
        segs_by_group[s[0]].append((i, s))
    cur = 0
    for gi in range(n_groups):
        seg_start_g[gi] = cur
        cur += len(segs_by_group[gi])
    seg_start_g[n_groups] = cur
    NSEG_G_MAX = max(len(v) for v in segs_by_group)
    mm_plan = {}
    for gi, g in enumerate(groups):
        for t in g:
            lst = []
            for i, (gi2, b, ch, t2) in [(i, s) for i, s in segs_by_group[gi]]:
                if t2 == t:
                    chunk_local = int(ch0_cell[gi, b]) - group_start[gi] + ch
                    lst.append((i - seg_start_g[gi], chunk_local))
            mm_plan[(gi, t)] = lst

    in_maps = []
    for c in range(N_CORES):
        sel = core_o == c
        s_c = s_o[sel]
        d_c = d_o[sel]
        gid_c = gid_o[sel]
        bank_c = bank_o[sel]
        bloc_c = bloc_o[sel]
        trel_c = trel_o[sel]
        # slot within cell: edges are sorted by (group, bank, tile) already
        ck = (gid_c * N_BANKS + bank_c)
        run_starts = np.concatenate(
            [[0], np.cumsum(np.bincount(ck, minlength=n_groups * N_BANKS))]
        )[:-1]
        slot = np.arange(len(s_c)) - run_starts[ck]
        chv = ch0_cell.ravel()[ck] + slot // P
        pv = (slot % P).astype(np.int64)

        segv = np.array(
            [seg_index[(int(gi), int(b), int(sl // P), int(t))]
             for gi, b, sl, t in zip(gid_c, bank_c, slot, trel_c)],
            dtype=np.int64,
        )
        erel = np.full((P, NSEG), -1.0, dtype=np.float32)
        drel = (d_c % NT - (trel_c * P)).astype(np.float32)
        erel[pv, segv] = drel
        erelb = erel.astype(BF16)

        idx16 = np.zeros((16, NCH * 8), dtype=np.int16)
        idx16[pv % 16, chv * 8 + pv // 16] = bloc_c.astype(np.int16)
        idx128 = np.tile(idx16, (8, 1))

        xpad = np.zeros((100 * P, F_PAD), dtype=np.float32)
        xpad[:NT, :F_IN] = x[c * NT : (c + 1) * NT, :]
        # phase-A blocks of 4 tiles: [blk, p, cb, 4*128 nodes]
        xT4 = (
            xpad.reshape(NBLK, 4, P, 5, P)
            .transpose(0, 4, 3, 1, 2)
            .reshape(NBLK * P, 5 * 4 * P)
            .astype(BF16)
        )
        W1b = np.zeros((F_PAD, HID), dtype=np.float32)
        W1b[:F_IN, :] = W1
        W2b = np.zeros((HID, HID), dtype=np.float32)
        W2b[:, :C] = W2
        iota = np.tile(np.arange(P, dtype=np.float32), (P, 1))

        dloc = dinv[c * NT : (c + 1) * NT]
        dinv_nat = np.zeros((P, N_TILES), dtype=np.float32)
        for t in range(N_TILES):
            tsz = min(P, NT - t * P)
            dinv_nat[:tsz, t] = dloc[t * P : t * P + tsz]
        dinvbcT = np.zeros((HID, NTP), dtype=np.float32)
        dinvbcT[:, :NT] = dloc[None, :]

        in_maps.append(
            dict(
                xT4=np.ascontiguousarray(xT4),
                W1b=W1b.astype(BF16),
                W2b=W2b.astype(BF16),
                b1=b1.reshape(HID, 1).astype(np.float32),
                b2bc=np.tile(b2.reshape(1, C), (P, 1)).astype(np.float32),
                iota=iota.astype(BF16),
                ident=np.eye(P, dtype=BF16),
                dinv_nat=dinv_nat,
                dinvbcT=dinvbcT.astype(BF16),
                idx16=idx128,
                erel=erelb,
            )
        )
    plan = dict(NCH=NCH, NSEG=NSEG, groups=groups,
                group_start=group_start, group_cw=group_cw,
                call_plan=call_plan, CWG_MAX=CWG_MAX,
                seg_start_g=seg_start_g, NSEG_G_MAX=NSEG_G_MAX,
                mm_plan=mm_plan)
    return in_maps, plan


def _build(plan):
    NCH = plan["NCH"]
    NSEG = plan["NSEG"]
    groups = plan["groups"]
    n_groups = len(groups)
    group_start = plan["group_start"]
    call_plan = plan["call_plan"]
    CWG_MAX = plan["CWG_MAX"]
    seg_start_g = plan["seg_start_g"]
    NSEG_G_MAX = plan["NSEG_G_MAX"]
    mm_plan = plan["mm_plan"]
    f32 = mybir.dt.float32
    bf16 = mybir.dt.bfloat16
    i16 = mybir.dt.int16

    # group index after which slice-A tiles (0..TILES_A-1) are fully done
    GA_LAST = TILES_A // GROUP - 1          # group 11 for GROUP=4
    assert (GA_LAST + 1) * GROUP == TILES_A

    nc = bacc.Bacc("TRN2", target_bir_lowering=False, num_devices=N_CORES,
                   num_swdge_queues=4, dynamic_dma_scratch_size=32768)
    xT4 = nc.declare_dram_parameter("xT4", [NBLK * P, 5 * 4 * P], bf16, isOutput=False)
    W1p = nc.declare_dram_parameter("W1b", [F_PAD, HID], bf16, isOutput=False)
    W2p = nc.declare_dram_parameter("W2b", [HID, HID], bf16, isOutput=False)
    b1p = nc.declare_dram_parameter("b1", [HID, 1], f32, isOutput=False)
    b2bc_p = nc.declare_dram_parameter("b2bc", [P, C], f32, isOutput=False)
    iota_p = nc.declare_dram_parameter("iota", [P, P], bf16, isOutput=False)
    ident_p = nc.declare_dram_parameter("ident", [P, P], bf16, isOutput=False)
    dnat_p = nc.declare_dram_parameter("dinv_nat", [P, N_TILES], f32, isOutput=False)
    dbcT_p = nc.declare_dram_parameter("dinvbcT", [HID, NTP], bf16, isOutput=False)
    idx_p = nc.declare_dram_parameter("idx16", [P, NCH * 8], i16, isOutput=False)
    erel_p = nc.declare_dram_parameter("erel", [P, NSEG], bf16, isOutput=False)
    out_p = nc.declare_dram_parameter("out", [NT, C], f32, isOutput=True)

    hloc = {}
    hfull = {}
    for layer in (1, 2):
        for sl, sz in (("A", SZA), ("B", SZB)):
            hloc[(layer, sl)] = nc.dram_tensor(f"h{layer}{sl}_local", [sz, TW], bf16)
            hfull[(layer, sl)] = nc.dram_tensor(
                f"h{layer}{sl}_full", [N_CORES * sz, TW], bf16, addr_space="Shared"
            )
    rg = [list(range(N_CORES))]

    with tile.TileContext(nc) as tc, ExitStack() as ctx:
        consts = ctx.enter_context(tc.tile_pool(name="consts", bufs=1))
        big = ctx.enter_context(tc.tile_pool(name="big", bufs=1))
        xpool = ctx.enter_context(tc.tile_pool(name="xpool", bufs=2))
        stpool = ctx.enter_context(tc.tile_pool(name="stpool", bufs=2))
        gpool = ctx.enter_context(tc.tile_pool(name="gpool", bufs=3))
        sspool = ctx.enter_context(tc.tile_pool(name="sspool", bufs=2))
        smalls = ctx.enter_context(tc.tile_pool(name="smalls", bufs=3))
        psA = ctx.enter_context(tc.tile_pool(name="psA", bufs=2, space="PSUM"))
        psT = ctx.enter_context(tc.tile_pool(name="psT", bufs=1, space="PSUM"))
        ps1 = ctx.enter_context(tc.tile_pool(name="ps1", bufs=2, space="PSUM"))

        # ---------- constants ----------
        idxt = consts.tile([P, NCH * 8], i16)
        erel_b = consts.tile([P, NSEG], bf16)
        iota_b = consts.tile([P, P], bf16)
        nc.sync.dma_start(out=iota_b[:], in_=iota_p[:])
        ident_b = consts.tile([P, P], bf16)
        nc.sync.dma_start(out=ident_b[:], in_=ident_p[:])
        W1bt = consts.tile([P, 5 * HID], bf16)
        W1b3 = W1bt[:].rearrange("p (c h) -> p c h", c=5)
        nc.sync.dma_start(out=W1b3, in_=W1p[:].rearrange("(c p) h -> p c h", c=5))
        W2bt = consts.tile([HID, HID], bf16)
        nc.sync.dma_start(out=W2bt[:], in_=W2p[:])
        b1t = consts.tile([HID, 1], f32)
        nc.sync.dma_start(out=b1t[:], in_=b1p[:])
        b2t = consts.tile([P, C], f32)
        nc.sync.dma_start(out=b2t[:], in_=b2bc_p[:])
        dnat = consts.tile([P, N_TILES], f32)
        nc.sync.dma_start(out=dnat[:], in_=dnat_p[:])
        dbcT = consts.tile([HID, NTP], bf16)
        nc.sync.dma_start(out=dbcT[:], in_=dbcT_p[:])

        # virgin-SBUF defense: zero the gather/ss rings once so no stale NaN
        # bit-pattern can reach a matmul through an unwritten byte.
        for _ in range(3):
            gz = gpool.tile([P, CWG_MAX * TW], bf16, tag="gbuf")
            nc.vector.memset(gz[:], 0.0)
        for _ in range(2):
            sz = sspool.tile([P, NSEG_G_MAX * P], bf16, tag="ss")
            nc.vector.memset(sz[:], 0.0)

        # local table (natural layout; layer-1 then overwritten per-tile by
        # the folded phase D to become the layer-2 table)
        gloc = big.tile([P, N_TILES * HID], bf16)
        gloc3 = gloc[:].rearrange("p (t w) -> p t w", t=N_TILES)
        nc.vector.memset(gloc[:], 0.0)
        # transposed relu'd layer-1 activations
        h1rT = big.tile([HID, NTP], bf16)

        def table_out_dma(layer, t0, ntiles):
            """DMA gloc tiles [t0, t0+ntiles) to the layer's A/B local slice."""
            assert t0 + ntiles <= TILES_A or t0 >= TILES_A
            sl = "A" if t0 < TILES_A else "B"
            base = t0 * P - (0 if sl == "A" else SZA)
            dst = hloc[(layer, sl)][base : base + ntiles * P, :]
            nc.sync.dma_start(
                out=dst.rearrange("(t p) w -> p t w", t=ntiles)[:, :, 0:HID],
                in_=gloc3[:, t0 : t0 + ntiles, :],
            )

        def ag(layer, sl):
            nc.gpsimd.collective_compute(
                "AllGather", mybir.AluOpType.bypass, replica_groups=rg,
                ins=[hloc[(layer, sl)][:]], outs=[hfull[(layer, sl)][:]],
            )

        # ---------- phase A: g1 = (x @ W1) * dinv, natural bf16 ----------
        with nc.named_scope("phaseA"):
            for blk in range(NBLK):
                xb = xpool.tile([P, 5 * 4 * P], bf16, tag="xb")
                nc.sync.dma_start(out=xb[:], in_=xT4[blk * P : (blk + 1) * P, :])
                xb3 = xb[:].rearrange("p (c n) -> p c n", c=5)
                pa = psA.tile([HID, 4 * P], f32, tag="pa")
                for cb in range(5):
                    nc.tensor.matmul(
                        out=pa[:], lhsT=W1b3[:, cb, :], rhs=xb3[:, cb, :],
                        start=(cb == 0), stop=(cb == 4),
                    )
                st = stpool.tile([HID, 4 * P], bf16, tag="st")
                nc.scalar.activation(
                    out=st[:], in_=pa[:],
                    func=mybir.ActivationFunctionType.Copy,
                )
                ntiles = 0
                for i in range(4):
                    t = blk * 4 + i
                    if t >= N_TILES:
                        break
                    ntiles += 1
                    tsz = min(P, NT - t * P)
                    pt = psT.tile([P, HID], bf16, tag="ptr")
                    nc.tensor.transpose(
                        out=pt[:], in_=st[:, i * P : (i + 1) * P],
                        identity=ident_b[:HID, :HID],
                    )
                    nc.scalar.activation(
                        out=gloc3[:tsz, t, :], in_=pt[:tsz, :],
                        func=mybir.ActivationFunctionType.Copy,
                        scale=dnat[:tsz, t : t + 1],
                    )
                if ntiles:
                    table_out_dma(1, blk * 4, ntiles)
                if blk * 4 + 4 == TILES_A:
                    ag(1, "A")
                    # conv-only constants: loaded here so they don't delay
                    # the phase-A x blocks or the slice-A AllGather
                    nc.sync.dma_start(out=idxt[:], in_=idx_p[:])
                    nc.sync.dma_start(out=erel_b[:], in_=erel_p[:])
            ag(1, "B")

        def conv(layer):
            hA = hfull[(layer, "A")]
            hB = hfull[(layer, "B")]
            bank_ap = {
                0: hA[0 : BANK_SZ[0], :],
                1: hA[BANK_SZ[0] : 2 * BANK_SZ[0], :],
                2: hB[0 : BANK_SZ[2], :],
                3: hB[BANK_SZ[2] : 2 * BANK_SZ[2], :],
            }
            gbufs = {}

            def gather_half(gi, half):
                if gi not in gbufs:
                    gb = gpool.tile([P, CWG_MAX * TW], bf16, tag="gbuf", name="gbuf")
                    gbufs[gi] = gb
                g3 = gbufs[gi][:].rearrange("p (c w) -> p c w", c=CWG_MAX)
                gs = group_start[gi]
                for (gg, b, ch0, cw, nidx) in call_plan:
                    if gg != gi or (b // 2) != half:
                        continue
                    loc = ch0 - gs
                    nc.gpsimd.dma_gather(
                        out_ap=g3[:, loc : loc + cw, :],
                        in_ap=bank_ap[b],
                        idxs_ap=idxt[:, ch0 * 8 : (ch0 + cw) * 8],
                        num_idxs=nidx,
                        num_idxs_reg=nidx,
                        elem_size=TW,
                        single_packet=SINGLE_PACKET,
                        queue_num=(b + gi) % 4,
                    )

            scope = f"conv{layer}"
            with nc.named_scope(scope):
                for gi, g in enumerate(groups):
                    if gi == 0:
                        for pf in range(min(2, n_groups)):
                            gather_half(pf, 0)
                    if gi + 2 < n_groups:
                        gather_half(gi + 2, 0)
                    gather_half(gi, 1)
                    gbuf = gbufs.pop(gi)
                    g3 = gbuf[:].rearrange("p (c w) -> p c w", c=CWG_MAX)
                    ss0 = seg_start_g[gi]
                    nseg_g = seg_start_g[gi + 1] - ss0
                    ss = sspool.tile([P, NSEG_G_MAX * P], bf16, tag="ss")
                    ss3 = ss[:].rearrange("p (c n) -> p c n", c=NSEG_G_MAX)
                    nc.vector.tensor_tensor(
                        out=ss3[:, :nseg_g, :],
                        in0=iota_b[:].unsqueeze(1).to_broadcast([P, nseg_g, P]),
                        in1=erel_b[:, ss0 : ss0 + nseg_g].unsqueeze(2).to_broadcast(
                            [P, nseg_g, P]
                        ),
                        op=mybir.AluOpType.is_equal,
                    )
                    if layer == 2:
                        lg = smalls.tile([P, len(g) * C], f32, tag="lgrp")
                        lg3 = lg[:].rearrange("p (t c) -> p t c", t=len(g))
                    for ti, t in enumerate(g):
                        t0 = t * P
                        tsz = min(P, NT - t0)
                        segs_t = mm_plan[(gi, t)]
                        n_mm = len(segs_t) + 1
                        if layer == 1:
                            pt = ps1.tile([HID, P], f32, tag="pt1")
                            nc.tensor.matmul(
                                out=pt[:], lhsT=gloc3[:, t, :], rhs=ident_b[:],
                                start=True, stop=(n_mm == 1),
                            )
                        else:
                            pt = ps1.tile([P, HID], f32, tag="pt2")
                            nc.tensor.matmul(
                                out=pt[:], lhsT=ident_b[:], rhs=gloc3[:, t, :],
                                start=True, stop=(n_mm == 1),
                            )
                        for k, (sl, chl) in enumerate(segs_t):
                            if layer == 1:
                                nc.tensor.matmul(
                                    out=pt[:],
                                    lhsT=g3[:, chl, 0:HID],
                                    rhs=ss3[:, sl, :],
                                    start=False, stop=(k == n_mm - 2),
                                )
                            else:
                                nc.tensor.matmul(
                                    out=pt[:],
                                    lhsT=ss3[:, sl, :],
                                    rhs=g3[:, chl, 0:HID],
                                    start=False, stop=(k == n_mm - 2),
                                )
                        if layer == 1:
                            nc.vector.tensor_tensor(
                                out=pt[:], in0=pt[:],
                                in1=dbcT[:, t0 : t0 + P],
                                op=mybir.AluOpType.mult,
                            )
                            nc.scalar.activation(
                                out=h1rT[:, t0 : t0 + P], in_=pt[:],
                                func=mybir.ActivationFunctionType.Relu,
                                bias=b1t[:],
                            )
                            # folded phase D: h2 = (relu(h1) @ W2) * dinv
                            pb = psT.tile([P, HID], f32, tag="pd")
                            nc.tensor.matmul(
                                out=pb[:], lhsT=h1rT[:, t0 : t0 + P], rhs=W2bt[:],
                                start=True, stop=True,
                            )
                            nc.scalar.activation(
                                out=gloc3[:tsz, t, :], in_=pb[:tsz, :],
                                func=mybir.ActivationFunctionType.Copy,
                                scale=dnat[:tsz, t : t + 1],
                            )
                        else:
                            nc.scalar.activation(
                                out=lg3[:tsz, ti, :], in_=pt[:tsz, 0:C],
                                func=mybir.ActivationFunctionType.Copy,
                                scale=dnat[:tsz, t : t + 1],
                            )
                    if layer == 1:
                        table_out_dma(2, g[0], len(g))
                        if gi == GA_LAST:
                            ag(2, "A")
                        elif gi == n_groups - 1:
                            ag(2, "B")
                    else:
                        ng = len(g)
                        nc.vector.tensor_tensor(
                            out=lg3, in0=lg3,
                            in1=b2t[:].unsqueeze(1).to_broadcast([P, ng, C]),
                            op=mybir.AluOpType.add,
                        )
                        negm = smalls.tile([P, len(g)], f32, tag="negm")
                        nc.vector.tensor_reduce(
                            out=negm[:], in_=lg3, axis=mybir.AxisListType.X,
                            op=mybir.AluOpType.max, negate=True,
                        )
                        nc.vector.tensor_tensor(
                            out=lg3, in0=lg3,
                            in1=negm[:].unsqueeze(2).to_broadcast([P, ng, C]),
                            op=mybir.AluOpType.add,
                        )
                        eb = smalls.tile([P, len(g) * C], f32, tag="eb")
                        nc.scalar.activation(
                            out=eb[:], in_=lg[:],
                            func=mybir.ActivationFunctionType.Exp,
                        )
                        sums = smalls.tile([P, len(g)], f32, tag="sums")
                        nc.vector.tensor_reduce(
                            out=sums[:],
                            in_=eb[:].rearrange("p (t c) -> p t c", t=ng),
                            axis=mybir.AxisListType.X, op=mybir.AluOpType.add,
                        )
                        lns = smalls.tile([P, len(g)], f32, tag="lns")
                        nc.scalar.activation(
                            out=lns[:], in_=sums[:],
                            func=mybir.ActivationFunctionType.Ln,
                        )
                        nc.vector.tensor_tensor(
                            out=lg3, in0=lg3,
                            in1=lns[:].unsqueeze(2).to_broadcast([P, ng, C]),
                            op=mybir.AluOpType.subtract,
                        )
                        for ti, t in enumerate(g):
                            t0 = t * P
                            tsz = min(P, NT - t0)
                            nc.sync.dma_start(
                                out=out_p[t0 : t0 + tsz, :], in_=lg3[:tsz, ti, :]
                            )

        conv(1)
        conv(2)

    nc.compile()
    return nc


def kernel(x, src, dst, W1, b1, W2, b2):
    in_maps, plan = _preprocess(x, src, dst, W1, b1, W2, b2)
    nc = _build(plan)
    res = run_bass_kernel_spmd(nc, in_maps, list(range(N_CORES)), trace=PROFILE)
    _LAST_RESULTS["exec_time_ns"] = getattr(res, "exec_time_ns", None)
    _LAST_RESULTS["profile_json"] = getattr(res, "profile_json", None)
    it = getattr(res, "instructions_and_trace", None)
    _LAST_RESULTS["trace_path"] = it[1] if it else None
    _LAST_RESULTS["scope_times"] = getattr(res, "per_core_scope_times", None)
    out = np.concatenate([res.results[c]["out"] for c in range(N_CORES)], axis=0)
    return out.astype(np.float32)
